# revision 7
# baseline (speedup 1.0000x reference)
"""Trainium2 Bass kernel for nn_EntailmentSelfAttention (8-core data parallel).

Problem (per batch element n, sentence s):
  q/k/v head projections (shared per-head weights), energy = q @ k.T per head,
  query-position masking, softmax over the QUERY axis, out = attn @ v,
  fc_out: out @ Wo.T + bo.

Mapping (one (n) per NeuronCore; S=2 sentences iterated inside):
  - All tensors kept "transposed" on-chip: head-dim/embed-dim on partitions,
    sequence on the free axis, so the softmax (over queries) reduces along the
    free axis.
  - The V projection is folded into fc_out on the host:
      out = concat_h((attn_h @ xv_h) @ Wv.T) @ Wo.T = concat_h(attn_h @ xv_h) @ Wcomb
  - The query mask enters the energy matmul as a 65th contraction row:
      kT_ext row64 = 1, qT_ext row64 = -3000 * (1 - mask_q); after the 1/sqrt(L)
      softmax scale this is -132.6 -> exp underflows to 0 exactly, matching the
      reference's -1e20 semantics.
  - Softmax denominators come for free from the Exp activation's accum_out; the
    1/rowsum normalization is folded into a per-l-row scale of xv before the
    attn @ xv matmul.
"""

import math

import numpy as np

import concourse.bass as bass
import concourse.tile as tile
from concourse import bacc, mybir
from concourse import bass_utils

# problem shapes (hardcoded per the harness contract)
N, S, L, E, H = 8, 2, 512, 1024, 16
D = E // H  # 64
DX = D + 1  # extended head dim (projection + mask/ones row)
P = 128
NCORES = 8
LC = L // P  # 4 l-chunks
BMASK = 3000.0
SCALE = 1.0 / math.sqrt(float(L))

F32 = mybir.dt.float32
# matmul compute dtype: float32r = fp32 data, single-pass PE mode (4x faster
# than true fp32). Flip to F32 if accuracy ever demands it.
MM_DT = mybir.dt.float32r


DT_MM = MM_DT  # dtype for all matmul-operand tiles / DRAM tensors


def build_kernel_body(tc, outs, ins):
    nc = tc.nc

    def _c(ap):
        # sim path: run_kernel declares DRAM as plain fp32; view as DT_MM
        return ap if ap.dtype == DT_MM else ap.bitcast(DT_MM)

    xq, xk, xv = _c(ins["xq"]), _c(ins["xk"]), ins["xv"]
    wq, wk, wcomb, bo = _c(ins["wq"]), _c(ins["wk"]), _c(ins["wcomb"]), ins["bo"]
    outT = outs["outT"]

    import contextlib

    with contextlib.ExitStack() as ctx:
        ek = ctx.enter_context
        consts = ek(tc.tile_pool(name="consts", bufs=1))
        stream = ek(tc.tile_pool(name="stream", bufs=4))
        qkpool = ek(tc.tile_pool(name="qk", bufs=2))
        xvpool = ek(tc.tile_pool(name="xv", bufs=1))
        xvspool = ek(tc.tile_pool(name="xvs", bufs=4))
        attnpool = ek(tc.tile_pool(name="attn", bufs=10))
        sumpool = ek(tc.tile_pool(name="sums", bufs=8))
        ztpool = ek(tc.tile_pool(name="zt", bufs=1))
        wspool = ek(tc.tile_pool(name="wstrip", bufs=2))
        outpool = ek(tc.tile_pool(name="out", bufs=3))
        pp_pf = ek(tc.tile_pool(name="pp_pf", bufs=2, space="PSUM"))
        pp_e = ek(tc.tile_pool(name="pp_e", bufs=2, space="PSUM"))
        pp_z = ek(tc.tile_pool(name="pp_z", bufs=1, space="PSUM"))

        # constants
        wq_sb = consts.tile([DX, DX], DT_MM, tag="wq")
        nc.sync.dma_start(wq_sb[:], wq[:])
        wk_sb = consts.tile([DX, DX], DT_MM, tag="wk")
        nc.sync.dma_start(wk_sb[:], wk[:])
        bo_sb = consts.tile([P, E // P], F32, tag="bo")
        nc.sync.dma_start(bo_sb[:], bo.rearrange("(jo p) -> p jo", p=P))

        GH = 4  # heads per group (PSUM: one z bank per head)
        ZTs = {}  # (s, h) -> [D, L] tile
        for s in range(S):
            # values for this sentence: [p, lc, e], l = lc*128 + p
            xv_sb = xvpool.tile([P, LC, E], F32, tag=f"xv{s % 2}")
            nc.sync.dma_start(xv_sb[:], xv[s].rearrange("(lo p) e -> p lo e", p=P))

            for g in range(H // GH):
                h0 = g * GH
                # projections: per head qT_ext/kT_ext [65, L]
                qes, kes = [], []
                for i in range(GH):
                    h = h0 + i
                    xq_t = stream.tile([DX, L], DT_MM, tag="xq_t")
                    nc.sync.dma_start(xq_t[:], xq[s, h])
                    pq = pp_pf.tile([DX, L], F32, tag="pf", name="pq")
                    nc.tensor.matmul(pq[:], wq_sb[:], xq_t[:], start=True, stop=True)
                    qe = qkpool.tile([DX, L], DT_MM, tag=f"qe{i}", name=f"qe_{s}_{h}")
                    nc.vector.tensor_copy(qe[:], pq[:])
                    qes.append(qe)

                    xk_t = stream.tile([DX, L], DT_MM, tag="xk_t")
                    nc.sync.dma_start(xk_t[:], xk[s, h])
                    pk = pp_pf.tile([DX, L], F32, tag="pf", name="pk")
                    nc.tensor.matmul(pk[:], wk_sb[:], xk_t[:], start=True, stop=True)
                    ke = qkpool.tile([DX, L], DT_MM, tag=f"ke{i}", name=f"ke_{s}_{h}")
                    nc.vector.tensor_copy(ke[:], pk[:])
                    kes.append(ke)

                zqs = [
                    pp_z.tile([D, L], F32, tag=f"z{i}", name=f"zq_{s}_{g}_{i}")
                    for i in range(GH)
                ]
                for c in range(LC):
                    rsum = sumpool.tile([P, GH], F32, tag="rsum")
                    ats = []
                    for i in range(GH):
                        ep = pp_e.tile([P, L], F32, tag="energy", name="ep")
                        nc.tensor.matmul(
                            ep[:],
                            kes[i][:, c * P:(c + 1) * P],
                            qes[i][:],
                            start=True,
                            stop=True,
                        )
                        at = attnpool.tile([P, L], DT_MM, tag="at", name="at")
                        nc.scalar.activation(
                            at[:],
                            ep[:],
                            mybir.ActivationFunctionType.Exp,
                            scale=SCALE,
                            accum_out=rsum[:, i:i + 1],
                        )
                        ats.append(at)
                    recip = sumpool.tile([P, GH], F32, tag="recip")
                    nc.vector.reciprocal(recip[:], rsum[:])
                    # xvs[p, i, d] = xv[p, c, (h0+i)*64+d] * recip[p, i]
                    xvs = xvspool.tile([P, GH, D], DT_MM, tag="xvs")
                    nc.vector.tensor_tensor(
                        xvs[:],
                        xv_sb[:, c, h0 * D:(h0 + GH) * D].rearrange(
                            "p (h d) -> p h d", d=D),
                        recip[:, :, None].to_broadcast((P, GH, D)),
                        mybir.AluOpType.mult,
                    )
                    for i in range(GH):
                        nc.tensor.matmul(
                            zqs[i][:],
                            xvs[:, i],
                            ats[i][:],
                            start=(c == 0),
                            stop=(c == LC - 1),
                        )
                for i in range(GH):
                    zt = ztpool.tile([D, L], DT_MM, tag=f"zt{s}_{h0 + i}",
                                     name=f"zt_{s}_{h0 + i}")
                    nc.vector.tensor_copy(zt[:], zqs[i][:])
                    ZTs[(s, h0 + i)] = zt

        # fc_out: outT[j, l] = sum_e Wcomb[e, j] * ZT[e, l] + bo[j]
        # contraction in K=64 chunks (one per head), Wcomb strips streamed
        for jt in range(E // P):
            ws = wspool.tile([D, H, P], DT_MM, tag="ws")
            nc.sync.dma_start(
                ws[:],
                wcomb[:, jt * P:(jt + 1) * P].rearrange("(ec p) j -> p ec j", p=D),
            )
            for s in range(S):
                fp = pp_pf.tile([P, L], F32, tag="pf", name="fp")
                for ec in range(H):
                    nc.tensor.matmul(
                        fp[:],
                        ws[:, ec, :],
                        ZTs[(s, ec)][:],
                        start=(ec == 0),
                        stop=(ec == H - 1),
                    )
                ot = outpool.tile([P, L], F32, tag="ot")
                nc.vector.tensor_scalar_add(ot[:], fp[:], bo_sb[:, jt:jt + 1])
                nc.sync.dma_start(outT[s, jt * P:(jt + 1) * P, :], ot[:])


def host_prepare(values, keys, query, mask, Wv, Wk, Wq, Wo, bo):
    """Build per-core input maps (host-side sharding + layout)."""
    values = np.ascontiguousarray(np.asarray(values, dtype=np.float32))
    keys = np.asarray(keys, dtype=np.float32)
    query = np.asarray(query, dtype=np.float32)
    mask = np.asarray(mask)
    Wv = np.asarray(Wv, dtype=np.float32)
    Wk = np.asarray(Wk, dtype=np.float32)
    Wq = np.asarray(Wq, dtype=np.float32)
    Wo = np.asarray(Wo, dtype=np.float32)
    bo = np.ascontiguousarray(np.asarray(bo, dtype=np.float32))

    qb = (-BMASK * (1.0 - mask[:, :, :, 0].astype(np.float32)))  # (N, S, L)
    qT = query.transpose(0, 1, 3, 2).reshape(N, S, H, D, L)
    kT = keys.transpose(0, 1, 3, 2).reshape(N, S, H, D, L)
    ones_row = np.ones((N, S, H, 1, L), np.float32)
    qb_row = np.broadcast_to(qb[:, :, None, None, :], (N, S, H, 1, L))
    xq = np.ascontiguousarray(np.concatenate([qT, qb_row], axis=3))  # (N,S,H,65,L)
    xk = np.ascontiguousarray(np.concatenate([kT, ones_row], axis=3))

    wq_ext = np.zeros((DX, DX), np.float32)
    wq_ext[:D, :D] = Wq.T
    wq_ext[D, D] = 1.0
    wk_ext = np.zeros((DX, DX), np.float32)
    wk_ext[:D, :D] = Wk.T
    wk_ext[D, D] = 1.0

    wcomb = np.zeros((E, E), np.float32)
    for h in range(H):
        wcomb[h * D:(h + 1) * D, :] = Wv.T @ Wo[:, h * D:(h + 1) * D].T
    wcomb = np.ascontiguousarray(wcomb)

    shared = {"wq": wq_ext, "wk": wk_ext, "wcomb": wcomb, "bo": bo}
    in_maps = []
    for n in range(NCORES):
        m = {"xq": xq[n], "xk": xk[n], "xv": values[n]}
        m.update(shared)
        in_maps.append(m)
    return in_maps


_NC_CACHE = None


def _get_program():
    global _NC_CACHE
    if _NC_CACHE is not None:
        return _NC_CACHE
    nc = bacc.Bacc("TRN2", target_bir_lowering=False, debug=False,
                   num_devices=NCORES)
    ins = {
        "xq": nc.dram_tensor("xq", (S, H, DX, L), DT_MM, kind="ExternalInput").ap(),
        "xk": nc.dram_tensor("xk", (S, H, DX, L), DT_MM, kind="ExternalInput").ap(),
        "xv": nc.dram_tensor("xv", (S, L, E), F32, kind="ExternalInput").ap(),
        "wq": nc.dram_tensor("wq", (DX, DX), DT_MM, kind="ExternalInput").ap(),
        "wk": nc.dram_tensor("wk", (DX, DX), DT_MM, kind="ExternalInput").ap(),
        "wcomb": nc.dram_tensor("wcomb", (E, E), DT_MM, kind="ExternalInput").ap(),
        "bo": nc.dram_tensor("bo", (E,), F32, kind="ExternalInput").ap(),
    }
    outs = {
        "outT": nc.dram_tensor("outT", (S, E, L), F32, kind="ExternalOutput").ap(),
    }
    with tile.TileContext(nc) as tc:
        build_kernel_body(tc, outs, ins)
    nc.compile()
    _NC_CACHE = nc
    return nc


def run(inputs: dict, trace: bool = False):
    """Run on 8 cores; returns (full_output, BassKernelResults)."""
    nc = _get_program()
    in_maps = host_prepare(**inputs)
    res = bass_utils.run_bass_kernel_spmd(
        nc, in_maps, core_ids=list(range(NCORES)), trace=trace,
    )
    out = np.empty((N, S, L, E), np.float32)
    for n in range(NCORES):
        out[n] = res.results[n]["outT"].transpose(0, 2, 1)
    return out, res


def kernel(**inputs) -> np.ndarray:
    out, _ = run(inputs, trace=False)
    return out


# revision 9
# speedup vs baseline: 1.3321x; 1.3321x over previous
"""Trainium2 Bass kernel for nn_EntailmentSelfAttention (8-core data parallel).

Problem (per batch element n, sentence s):
  q/k/v head projections (shared per-head weights), energy = q @ k.T per head,
  query-position masking, softmax over the QUERY axis, out = attn @ v,
  fc_out: out @ Wo.T + bo.

Mapping (one (n) per NeuronCore; S=2 sentences iterated inside):
  - All tensors kept "transposed" on-chip: head-dim/embed-dim on partitions,
    sequence on the free axis, so the softmax (over queries) reduces along the
    free axis.
  - The V projection is folded into fc_out on the host:
      out = concat_h((attn_h @ xv_h) @ Wv.T) @ Wo.T = concat_h(attn_h @ xv_h) @ Wcomb
  - The query mask enters the energy matmul as a 65th contraction row:
      kT_ext row64 = 1, qT_ext row64 = -3000 * (1 - mask_q); after the 1/sqrt(L)
      softmax scale this is -132.6 -> exp underflows to 0 exactly, matching the
      reference's -1e20 semantics.
  - Softmax denominators come for free from the Exp activation's accum_out; the
    1/rowsum normalization is folded into a per-l-row scale of xv before the
    attn @ xv matmul.
"""

import math

import numpy as np

import concourse.bass as bass
import concourse.tile as tile
from concourse import bacc, mybir
from concourse import bass_utils

# problem shapes (hardcoded per the harness contract)
N, S, L, E, H = 8, 2, 512, 1024, 16
D = E // H  # 64
DX = D + 1  # extended head dim (projection + mask/ones row)
P = 128
NCORES = 8
LC = L // P  # 4 l-chunks
BMASK = 3000.0
SCALE = 1.0 / math.sqrt(float(L))

F32 = mybir.dt.float32
BF16 = mybir.dt.bfloat16
# matmul compute dtype: float32r = fp32 data, single-pass PE mode (4x faster
# than true fp32). Flip to F32 if accuracy ever demands it.
MM_DT = mybir.dt.float32r


DT_MM = MM_DT  # dtype for all matmul-operand tiles / DRAM tensors


def build_kernel_body(tc, outs, ins):
    nc = tc.nc

    def _c(ap):
        # sim path: run_kernel declares DRAM as plain fp32; view as DT_MM
        return ap if ap.dtype == DT_MM else ap.bitcast(DT_MM)

    xq, xk, xv = _c(ins["xq"]), _c(ins["xk"]), ins["xv"]
    wq, wk, wcomb, bo = _c(ins["wq"]), _c(ins["wk"]), _c(ins["wcomb"]), ins["bo"]
    outT = outs["outT"]

    import contextlib

    with contextlib.ExitStack() as ctx:
        ek = ctx.enter_context
        consts = ek(tc.tile_pool(name="consts", bufs=1))
        stream = ek(tc.tile_pool(name="stream", bufs=4))
        qkpool = ek(tc.tile_pool(name="qk", bufs=2))
        xvpool = ek(tc.tile_pool(name="xv", bufs=1))
        xvspool = ek(tc.tile_pool(name="xvs", bufs=4))
        attnpool = ek(tc.tile_pool(name="attn", bufs=10))
        sumpool = ek(tc.tile_pool(name="sums", bufs=8))
        ztpool = ek(tc.tile_pool(name="zt", bufs=1))
        outpool = ek(tc.tile_pool(name="out", bufs=3))
        pp_pf = ek(tc.tile_pool(name="pp_pf", bufs=2, space="PSUM"))
        pp_e = ek(tc.tile_pool(name="pp_e", bufs=4, space="PSUM"))
        pp_z = ek(tc.tile_pool(name="pp_z", bufs=1, space="PSUM"))

        # constants
        wq_sb = consts.tile([DX, DX], DT_MM, tag="wq")
        nc.sync.dma_start(wq_sb[:], wq[:])
        wk_sb = consts.tile([DX, DX], DT_MM, tag="wk")
        nc.sync.dma_start(wk_sb[:], wk[:])
        wcomb_sb = consts.tile([P, E // P, E], DT_MM, tag="wcomb")
        nc.sync.dma_start(wcomb_sb[:], wcomb.rearrange("(eo p) j -> p eo j", p=P))
        bo_sb = consts.tile([P, E // P], F32, tag="bo")
        nc.sync.dma_start(bo_sb[:], bo.rearrange("(jo p) -> p jo", p=P))

        GH = 4  # heads per group (PSUM: one z bank per head pair)
        for s in range(S):
            # values for this sentence: [p, lc, e], l = lc*128 + p
            xv_sb = xvpool.tile([P, LC, E], F32, tag=f"xv{s % 2}")
            nc.sync.dma_start(xv_sb[:], xv[s].rearrange("(lo p) e -> p lo e", p=P))

            ZT = ztpool.tile([P, E // P, L], DT_MM, tag=f"zt{s % 2}", name=f"zt_{s}")
            for g in range(H // GH):
                h0 = g * GH
                # projections: per head qT_ext/kT_ext [65, L]; group q-projs
                # then k-projs so the stationary weight reloads only once.
                qes, kes = [], []
                for i in range(GH):
                    h = h0 + i
                    xq_t = stream.tile([DX, L], DT_MM, tag="xq_t")
                    nc.sync.dma_start(xq_t[:], xq[s, h])
                    pq = pp_pf.tile([DX, L], F32, tag="pf", name="pq")
                    nc.tensor.matmul(pq[:], wq_sb[:], xq_t[:], start=True, stop=True)
                    qe = qkpool.tile([DX, L], DT_MM, tag=f"qe{i}", name=f"qe_{s}_{h}")
                    nc.vector.tensor_copy(qe[:], pq[:])
                    qes.append(qe)
                for i in range(GH):
                    h = h0 + i
                    xk_t = stream.tile([DX, L], DT_MM, tag="xk_t")
                    nc.sync.dma_start(xk_t[:], xk[s, h])
                    pk = pp_pf.tile([DX, L], F32, tag="pf", name="pk")
                    nc.tensor.matmul(pk[:], wk_sb[:], xk_t[:], start=True, stop=True)
                    ke = qkpool.tile([DX, L], DT_MM, tag=f"ke{i}", name=f"ke_{s}_{h}")
                    nc.vector.tensor_copy(ke[:], pk[:])
                    kes.append(ke)

                # one z psum bank per head PAIR: head A -> partitions 0:64,
                # head B -> partitions 64:128 (separate accumulation groups).
                zps = [
                    pp_z.tile([P, L], F32, tag=f"z{p_}", name=f"zp_{s}_{g}_{p_}")
                    for p_ in range(GH // 2)
                ]
                for c in range(LC):
                    rsum = sumpool.tile([P, GH], F32, tag="rsum")
                    ats = []
                    for i in range(GH):
                        ep = pp_e.tile([P, L], F32, tag="energy", name="ep")
                        nc.tensor.matmul(
                            ep[:],
                            kes[i][:, c * P:(c + 1) * P],
                            qes[i][:],
                            start=True,
                            stop=True,
                        )
                        at = attnpool.tile([P, L], BF16, tag="at", name="at")
                        nc.scalar.activation(
                            at[:],
                            ep[:],
                            mybir.ActivationFunctionType.Exp,
                            scale=SCALE,
                            accum_out=rsum[:, i:i + 1],
                        )
                        ats.append(at)
                    recip = sumpool.tile([P, GH], F32, tag="recip")
                    nc.vector.reciprocal(recip[:], rsum[:])
                    # xvs[p, i, d] = xv[p, c, (h0+i)*64+d] * recip[p, i]
                    xvs = xvspool.tile([P, GH, D], BF16, tag="xvs")
                    nc.vector.tensor_tensor(
                        xvs[:],
                        xv_sb[:, c, h0 * D:(h0 + GH) * D].rearrange(
                            "p (h d) -> p h d", d=D),
                        recip[:, :, None].to_broadcast((P, GH, D)),
                        mybir.AluOpType.mult,
                    )
                    for i in range(GH):
                        zp = zps[i // 2]
                        lo = (i % 2) * D
                        nc.tensor.matmul(
                            zp[lo:lo + D, :],
                            xvs[:, i],
                            ats[i][:],
                            start=(c == 0),
                            stop=(c == LC - 1),
                            skip_group_check=True,
                        )
                for p_ in range(GH // 2):
                    nc.vector.tensor_copy(ZT[:, g * (GH // 2) + p_, :], zps[p_][:])

            # fc_out: outT[j, l] = sum_e Wcomb[e, j] * ZT[e, l] + bo[j]
            for jt in range(E // P):
                fp = pp_pf.tile([P, L], F32, tag="pf", name="fp")
                for eo in range(E // P):
                    nc.tensor.matmul(
                        fp[:],
                        wcomb_sb[:, eo, jt * P:(jt + 1) * P],
                        ZT[:, eo, :],
                        start=(eo == 0),
                        stop=(eo == E // P - 1),
                    )
                ot = outpool.tile([P, L], F32, tag="ot")
                nc.vector.tensor_scalar_add(ot[:], fp[:], bo_sb[:, jt:jt + 1])
                nc.sync.dma_start(outT[s, jt * P:(jt + 1) * P, :], ot[:])


def host_prepare(values, keys, query, mask, Wv, Wk, Wq, Wo, bo):
    """Build per-core input maps (host-side sharding + layout)."""
    values = np.ascontiguousarray(np.asarray(values, dtype=np.float32))
    keys = np.asarray(keys, dtype=np.float32)
    query = np.asarray(query, dtype=np.float32)
    mask = np.asarray(mask)
    Wv = np.asarray(Wv, dtype=np.float32)
    Wk = np.asarray(Wk, dtype=np.float32)
    Wq = np.asarray(Wq, dtype=np.float32)
    Wo = np.asarray(Wo, dtype=np.float32)
    bo = np.ascontiguousarray(np.asarray(bo, dtype=np.float32))

    qb = (-BMASK * (1.0 - mask[:, :, :, 0].astype(np.float32)))  # (N, S, L)
    qT = query.transpose(0, 1, 3, 2).reshape(N, S, H, D, L)
    kT = keys.transpose(0, 1, 3, 2).reshape(N, S, H, D, L)
    ones_row = np.ones((N, S, H, 1, L), np.float32)
    qb_row = np.broadcast_to(qb[:, :, None, None, :], (N, S, H, 1, L))
    xq = np.ascontiguousarray(np.concatenate([qT, qb_row], axis=3))  # (N,S,H,65,L)
    xk = np.ascontiguousarray(np.concatenate([kT, ones_row], axis=3))

    wq_ext = np.zeros((DX, DX), np.float32)
    wq_ext[:D, :D] = Wq.T
    wq_ext[D, D] = 1.0
    wk_ext = np.zeros((DX, DX), np.float32)
    wk_ext[:D, :D] = Wk.T
    wk_ext[D, D] = 1.0

    wcomb = np.zeros((E, E), np.float32)
    for h in range(H):
        wcomb[h * D:(h + 1) * D, :] = Wv.T @ Wo[:, h * D:(h + 1) * D].T
    wcomb = np.ascontiguousarray(wcomb)

    shared = {"wq": wq_ext, "wk": wk_ext, "wcomb": wcomb, "bo": bo}
    in_maps = []
    for n in range(NCORES):
        m = {"xq": xq[n], "xk": xk[n], "xv": values[n]}
        m.update(shared)
        in_maps.append(m)
    return in_maps


_NC_CACHE = None


def _get_program():
    global _NC_CACHE
    if _NC_CACHE is not None:
        return _NC_CACHE
    nc = bacc.Bacc("TRN2", target_bir_lowering=False, debug=False,
                   num_devices=NCORES)
    ins = {
        "xq": nc.dram_tensor("xq", (S, H, DX, L), DT_MM, kind="ExternalInput").ap(),
        "xk": nc.dram_tensor("xk", (S, H, DX, L), DT_MM, kind="ExternalInput").ap(),
        "xv": nc.dram_tensor("xv", (S, L, E), F32, kind="ExternalInput").ap(),
        "wq": nc.dram_tensor("wq", (DX, DX), DT_MM, kind="ExternalInput").ap(),
        "wk": nc.dram_tensor("wk", (DX, DX), DT_MM, kind="ExternalInput").ap(),
        "wcomb": nc.dram_tensor("wcomb", (E, E), DT_MM, kind="ExternalInput").ap(),
        "bo": nc.dram_tensor("bo", (E,), F32, kind="ExternalInput").ap(),
    }
    outs = {
        "outT": nc.dram_tensor("outT", (S, E, L), F32, kind="ExternalOutput").ap(),
    }
    with tile.TileContext(nc) as tc:
        build_kernel_body(tc, outs, ins)
    nc.compile()
    _NC_CACHE = nc
    return nc


def run(inputs: dict, trace: bool = False):
    """Run on 8 cores; returns (full_output, BassKernelResults)."""
    nc = _get_program()
    in_maps = host_prepare(**inputs)
    res = bass_utils.run_bass_kernel_spmd(
        nc, in_maps, core_ids=list(range(NCORES)), trace=trace,
    )
    out = np.empty((N, S, L, E), np.float32)
    for n in range(NCORES):
        out[n] = res.results[n]["outT"].transpose(0, 2, 1)
    return out, res


def kernel(**inputs) -> np.ndarray:
    out, _ = run(inputs, trace=False)
    return out


# revision 11
# speedup vs baseline: 1.7384x; 1.3051x over previous
"""Trainium2 Bass kernel for nn_EntailmentSelfAttention (8-core data parallel).

Problem (per batch element n, sentence s):
  q/k/v head projections (shared per-head weights), energy = q @ k.T per head,
  query-position masking, softmax over the QUERY axis, out = attn @ v,
  fc_out: out @ Wo.T + bo.

Mapping (one (n) per NeuronCore; S=2 sentences iterated inside):
  - All tensors kept "transposed" on-chip: head-dim/embed-dim on partitions,
    sequence on the free axis, so the softmax (over queries) reduces along the
    free axis.
  - The V projection is folded into fc_out on the host:
      out = concat_h((attn_h @ xv_h) @ Wv.T) @ Wo.T = concat_h(attn_h @ xv_h) @ Wcomb
  - The query mask enters the energy matmul as a 65th contraction row:
      kT_ext row64 = 1, qT_ext row64 = -3000 * (1 - mask_q); after the 1/sqrt(L)
      softmax scale this is -132.6 -> exp underflows to 0 exactly, matching the
      reference's -1e20 semantics.
  - Softmax denominators come for free from the Exp activation's accum_out; the
    1/rowsum normalization is folded into a per-l-row scale of xv before the
    attn @ xv matmul.
"""

import math

import numpy as np

import concourse.bass as bass
import concourse.tile as tile
from concourse import bacc, mybir
from concourse import bass_utils

# problem shapes (hardcoded per the harness contract)
N, S, L, E, H = 8, 2, 512, 1024, 16
D = E // H  # 64
DX = D + 1  # extended head dim (projection + mask/ones row)
P = 128
NCORES = 8
LC = L // P  # 4 l-chunks
BMASK = 3000.0
QP_MIN = 256  # min compacted query length (keeps fp32r matmuls in 1-cyc mode)
SCALE = 1.0 / math.sqrt(float(L))

F32 = mybir.dt.float32
BF16 = mybir.dt.bfloat16
# matmul compute dtype: float32r = fp32 data, single-pass PE mode (4x faster
# than true fp32). Flip to F32 if accuracy ever demands it.
MM_DT = mybir.dt.float32r


DT_MM = MM_DT  # dtype for all matmul-operand tiles / DRAM tensors


def build_kernel_body(tc, outs, ins, QP):
    nc = tc.nc

    def _c(ap):
        # sim path: run_kernel declares DRAM as plain fp32; view as DT_MM
        return ap if ap.dtype == DT_MM else ap.bitcast(DT_MM)

    xq, xk, xv = _c(ins["xq"]), _c(ins["xk"]), ins["xv"]
    wq, wk, wcomb, bo = _c(ins["wq"]), _c(ins["wk"]), _c(ins["wcomb"]), ins["bo"]
    outT = outs["outT"]

    import contextlib

    with contextlib.ExitStack() as ctx:
        ek = ctx.enter_context
        consts = ek(tc.tile_pool(name="consts", bufs=1))
        stream = ek(tc.tile_pool(name="stream", bufs=4))
        qkpool = ek(tc.tile_pool(name="qk", bufs=2))
        xvpool = ek(tc.tile_pool(name="xv", bufs=1))
        xvspool = ek(tc.tile_pool(name="xvs", bufs=4))
        attnpool = ek(tc.tile_pool(name="attn", bufs=10))
        sumpool = ek(tc.tile_pool(name="sums", bufs=8))
        ztpool = ek(tc.tile_pool(name="zt", bufs=1))
        outpool = ek(tc.tile_pool(name="out", bufs=3))
        pp_pf = ek(tc.tile_pool(name="pp_pf", bufs=2, space="PSUM"))
        pp_e = ek(tc.tile_pool(name="pp_e", bufs=4, space="PSUM"))
        pp_z = ek(tc.tile_pool(name="pp_z", bufs=1, space="PSUM"))

        # constants
        wq_sb = consts.tile([DX, DX], DT_MM, tag="wq")
        nc.sync.dma_start(wq_sb[:], wq[:])
        wk_sb = consts.tile([DX, DX], DT_MM, tag="wk")
        nc.sync.dma_start(wk_sb[:], wk[:])
        wcomb_sb = consts.tile([P, E // P, E], DT_MM, tag="wcomb")
        nc.sync.dma_start(wcomb_sb[:], wcomb.rearrange("(eo p) j -> p eo j", p=P))
        bo_sb = consts.tile([P, E // P], F32, tag="bo")
        nc.sync.dma_start(bo_sb[:], bo.rearrange("(jo p) -> p jo", p=P))

        GH = 4  # heads per group (PSUM: one z bank per head pair)
        for s in range(S):
            # values for this sentence: [p, lc, e], l = lc*128 + p
            xv_sb = xvpool.tile([P, LC, E], F32, tag=f"xv{s % 2}")
            nc.sync.dma_start(xv_sb[:], xv[s].rearrange("(lo p) e -> p lo e", p=P))

            ZT = ztpool.tile([P, E // P, QP], DT_MM, tag=f"zt{s % 2}", name=f"zt_{s}")
            for g in range(H // GH):
                h0 = g * GH
                # projections: per head qT_ext/kT_ext; group q-projs then
                # k-projs so the stationary weight reloads only once.
                qes, kes = [], []
                for i in range(GH):
                    h = h0 + i
                    xq_t = stream.tile([DX, QP], DT_MM, tag="xq_t")
                    nc.sync.dma_start(xq_t[:], xq[s, h])
                    pq = pp_pf.tile([DX, QP], F32, tag="pf", name="pq")
                    nc.tensor.matmul(pq[:], wq_sb[:], xq_t[:], start=True, stop=True)
                    qe = qkpool.tile([DX, QP], BF16, tag=f"qe{i}", name=f"qe_{s}_{h}")
                    nc.vector.tensor_copy(qe[:], pq[:])
                    qes.append(qe)
                for i in range(GH):
                    h = h0 + i
                    xk_t = stream.tile([DX, L], DT_MM, tag="xk_t")
                    nc.sync.dma_start(xk_t[:], xk[s, h])
                    pk = pp_pf.tile([DX, L], F32, tag="pf", name="pk")
                    nc.tensor.matmul(pk[:], wk_sb[:], xk_t[:], start=True, stop=True)
                    ke = qkpool.tile([DX, L], BF16, tag=f"ke{i}", name=f"ke_{s}_{h}")
                    nc.vector.tensor_copy(ke[:], pk[:])
                    kes.append(ke)

                # one z psum bank per head PAIR: head A -> partitions 0:64,
                # head B -> partitions 64:128 (separate accumulation groups).
                zps = [
                    pp_z.tile([P, QP], F32, tag=f"z{p_}", name=f"zp_{s}_{g}_{p_}")
                    for p_ in range(GH // 2)
                ]
                for c in range(LC):
                    rsum = sumpool.tile([P, GH], F32, tag="rsum")
                    ats = []
                    for i in range(GH):
                        ep = pp_e.tile([P, QP], F32, tag="energy", name="ep")
                        nc.tensor.matmul(
                            ep[:],
                            kes[i][:, c * P:(c + 1) * P],
                            qes[i][:],
                            start=True,
                            stop=True,
                        )
                        at = attnpool.tile([P, QP], BF16, tag="at", name="at")
                        if i % 2 == 0:
                            # rowsum on the scalar engine (fused accumulate)
                            nc.scalar.activation(
                                at[:],
                                ep[:],
                                mybir.ActivationFunctionType.Exp,
                                scale=SCALE,
                                accum_out=rsum[:, i:i + 1],
                            )
                        else:
                            # rowsum on the vector engine (load balance)
                            nc.scalar.activation(
                                at[:],
                                ep[:],
                                mybir.ActivationFunctionType.Exp,
                                scale=SCALE,
                            )
                            nc.vector.tensor_reduce(
                                rsum[:, i:i + 1],
                                at[:],
                                axis=mybir.AxisListType.X,
                                op=mybir.AluOpType.add,
                            )
                        ats.append(at)
                    recip = sumpool.tile([P, GH], F32, tag="recip")
                    nc.vector.reciprocal(recip[:], rsum[:])
                    # xvs[p, i, d] = xv[p, c, (h0+i)*64+d] * recip[p, i]
                    xvs = xvspool.tile([P, GH, D], BF16, tag="xvs")
                    nc.vector.tensor_tensor(
                        xvs[:],
                        xv_sb[:, c, h0 * D:(h0 + GH) * D].rearrange(
                            "p (h d) -> p h d", d=D),
                        recip[:, :, None].to_broadcast((P, GH, D)),
                        mybir.AluOpType.mult,
                    )
                    for i in range(GH):
                        zp = zps[i // 2]
                        lo = (i % 2) * D
                        nc.tensor.matmul(
                            zp[lo:lo + D, :],
                            xvs[:, i],
                            ats[i][:],
                            start=(c == 0),
                            stop=(c == LC - 1),
                            skip_group_check=True,
                        )
                for p_ in range(GH // 2):
                    nc.vector.tensor_copy(ZT[:, g * (GH // 2) + p_, :], zps[p_][:])

            # fc_out: outT[j, l] = sum_e Wcomb[e, j] * ZT[e, l] + bo[j]
            for jt in range(E // P):
                fp = pp_pf.tile([P, QP], F32, tag="pf", name="fp")
                for eo in range(E // P):
                    nc.tensor.matmul(
                        fp[:],
                        wcomb_sb[:, eo, jt * P:(jt + 1) * P],
                        ZT[:, eo, :],
                        start=(eo == 0),
                        stop=(eo == E // P - 1),
                    )
                ot = outpool.tile([P, QP], F32, tag="ot")
                nc.vector.tensor_scalar_add(ot[:], fp[:], bo_sb[:, jt:jt + 1])
                nc.sync.dma_start(outT[s, jt * P:(jt + 1) * P, :], ot[:])


def host_prepare(values, keys, query, mask, Wv, Wk, Wq, Wo, bo):
    """Host-side sharding + layout + query compaction.

    Returns (in_maps, QP, order, cnt, bo_np). Masked query positions are
    dropped entirely (their output is just bo); the surviving queries are
    compacted to the front and padded to QP columns. Pad columns carry a
    -BMASK bias row so their exp is exactly 0 (excluded from denominators).
    """
    values = np.ascontiguousarray(np.asarray(values, dtype=np.float32))
    keys = np.asarray(keys, dtype=np.float32)
    query = np.asarray(query, dtype=np.float32)
    mask = np.asarray(mask)
    Wv = np.asarray(Wv, dtype=np.float32)
    Wk = np.asarray(Wk, dtype=np.float32)
    Wq = np.asarray(Wq, dtype=np.float32)
    Wo = np.asarray(Wo, dtype=np.float32)
    bo_np = np.ascontiguousarray(np.asarray(bo, dtype=np.float32))

    keep = mask[:, :, :, 0] != 0  # (N, S, L) True = query position survives
    cnt = keep.sum(-1)  # (N, S)
    QP = int(np.ceil(max(int(cnt.max()), 1) / 64) * 64)
    QP = max(QP, QP_MIN)
    QP = min(QP, L)
    # stable partition: surviving query indices first
    order = np.argsort(~keep, axis=-1, kind="stable")  # (N, S, L)

    qT = query.transpose(0, 1, 3, 2).reshape(N, S, H, D, L)
    kT = keys.transpose(0, 1, 3, 2).reshape(N, S, H, D, L)

    # gather+pad queries: (N, S, H, D, QP)
    gidx = order[:, :, :QP]  # (N, S, QP)
    qTc = np.take_along_axis(
        qT, gidx[:, :, None, None, :].repeat(H, 2).repeat(D, 3), axis=4)
    pad = np.arange(QP)[None, None, :] >= cnt[:, :, None]  # (N, S, QP)
    qTc[pad[:, :, None, None, :].repeat(H, 2).repeat(D, 3)] = 0.0
    qb_row = np.where(pad, np.float32(-BMASK), np.float32(0.0)).astype(np.float32)
    xq = np.ascontiguousarray(
        np.concatenate([qTc, qb_row[:, :, None, None, :].repeat(H, 2)], axis=3))

    ones_row = np.ones((N, S, H, 1, L), np.float32)
    xk = np.ascontiguousarray(np.concatenate([kT, ones_row], axis=3))

    wq_ext = np.zeros((DX, DX), np.float32)
    wq_ext[:D, :D] = Wq.T
    wq_ext[D, D] = 1.0
    wk_ext = np.zeros((DX, DX), np.float32)
    wk_ext[:D, :D] = Wk.T
    wk_ext[D, D] = 1.0

    wcomb = np.zeros((E, E), np.float32)
    for h in range(H):
        wcomb[h * D:(h + 1) * D, :] = Wv.T @ Wo[:, h * D:(h + 1) * D].T
    wcomb = np.ascontiguousarray(wcomb)

    shared = {"wq": wq_ext, "wk": wk_ext, "wcomb": wcomb, "bo": bo_np}
    in_maps = []
    for n in range(NCORES):
        m = {"xq": xq[n], "xk": xk[n], "xv": values[n]}
        m.update(shared)
        in_maps.append(m)
    return in_maps, QP, order, cnt, bo_np


_NC_CACHE = {}


def _get_program(QP):
    nc = _NC_CACHE.get(QP)
    if nc is not None:
        return nc
    nc = bacc.Bacc("TRN2", target_bir_lowering=False, debug=False,
                   num_devices=NCORES)
    ins = {
        "xq": nc.dram_tensor("xq", (S, H, DX, QP), DT_MM, kind="ExternalInput").ap(),
        "xk": nc.dram_tensor("xk", (S, H, DX, L), DT_MM, kind="ExternalInput").ap(),
        "xv": nc.dram_tensor("xv", (S, L, E), F32, kind="ExternalInput").ap(),
        "wq": nc.dram_tensor("wq", (DX, DX), DT_MM, kind="ExternalInput").ap(),
        "wk": nc.dram_tensor("wk", (DX, DX), DT_MM, kind="ExternalInput").ap(),
        "wcomb": nc.dram_tensor("wcomb", (E, E), DT_MM, kind="ExternalInput").ap(),
        "bo": nc.dram_tensor("bo", (E,), F32, kind="ExternalInput").ap(),
    }
    outs = {
        "outT": nc.dram_tensor("outT", (S, E, QP), F32, kind="ExternalOutput").ap(),
    }
    with tile.TileContext(nc) as tc:
        build_kernel_body(tc, outs, ins, QP)
    nc.compile()
    _NC_CACHE[QP] = nc
    return nc


def run(inputs: dict, trace: bool = False):
    """Run on 8 cores; returns (full_output, BassKernelResults)."""
    in_maps, QP, order, cnt, bo_np = host_prepare(**inputs)
    nc = _get_program(QP)
    res = bass_utils.run_bass_kernel_spmd(
        nc, in_maps, core_ids=list(range(NCORES)), trace=trace,
    )
    out = np.empty((N, S, L, E), np.float32)
    out[:] = bo_np  # masked query rows: attention output is 0, fc adds bo
    for n in range(NCORES):
        oT = res.results[n]["outT"]  # (S, E, QP)
        for s in range(S):
            c = int(cnt[n, s])
            if c:
                out[n, s, order[n, s, :c], :] = oT[s, :, :c].T
    return out, res


def kernel(**inputs) -> np.ndarray:
    out, _ = run(inputs, trace=False)
    return out


# revision 12
# speedup vs baseline: 1.9299x; 1.1101x over previous
"""Trainium2 Bass kernel for nn_EntailmentSelfAttention (8-core data parallel).

Problem (per batch element n, sentence s):
  q/k/v head projections (shared per-head weights), energy = q @ k.T per head,
  query-position masking, softmax over the QUERY axis, out = attn @ v,
  fc_out: out @ Wo.T + bo.

Mapping (one (n) per NeuronCore; S=2 sentences iterated inside):
  - All tensors kept "transposed" on-chip: head-dim/embed-dim on partitions,
    sequence on the free axis, so the softmax (over queries) reduces along the
    free axis.
  - The V projection is folded into fc_out on the host:
      out = concat_h((attn_h @ xv_h) @ Wv.T) @ Wo.T = concat_h(attn_h @ xv_h) @ Wcomb
  - The query mask enters the energy matmul as a 65th contraction row:
      kT_ext row64 = 1, qT_ext row64 = -3000 * (1 - mask_q); after the 1/sqrt(L)
      softmax scale this is -132.6 -> exp underflows to 0 exactly, matching the
      reference's -1e20 semantics.
  - Softmax denominators come for free from the Exp activation's accum_out; the
    1/rowsum normalization is folded into a per-l-row scale of xv before the
    attn @ xv matmul.
"""

import math

import numpy as np

import concourse.bass as bass
import concourse.tile as tile
from concourse import bacc, mybir
from concourse import bass_utils

# problem shapes (hardcoded per the harness contract)
N, S, L, E, H = 8, 2, 512, 1024, 16
D = E // H  # 64
DX = D + 1  # extended head dim (projection + mask/ones row)
P = 128
NCORES = 8
LC = L // P  # 4 l-chunks
BMASK = 3000.0
QP_MIN = 256  # min compacted query length (keeps fp32r matmuls in 1-cyc mode)
SCALE = 1.0 / math.sqrt(float(L))

F32 = mybir.dt.float32
BF16 = mybir.dt.bfloat16
# matmul compute dtype: float32r = fp32 data, single-pass PE mode (4x faster
# than true fp32). Flip to F32 if accuracy ever demands it.
MM_DT = mybir.dt.float32r


DT_MM = MM_DT  # dtype for all matmul-operand tiles / DRAM tensors


def build_kernel_body(tc, outs, ins, QP):
    nc = tc.nc

    def _c(ap):
        # sim path: run_kernel declares DRAM as plain fp32; view as DT_MM
        return ap if ap.dtype == DT_MM else ap.bitcast(DT_MM)

    xq, xk, xv = _c(ins["xq"]), _c(ins["xk"]), ins["xv"]
    wq, wk, wcomb, bo = _c(ins["wq"]), _c(ins["wk"]), _c(ins["wcomb"]), ins["bo"]
    outT = outs["outT"]

    import contextlib

    with contextlib.ExitStack() as ctx:
        ek = ctx.enter_context
        consts = ek(tc.tile_pool(name="consts", bufs=1))
        stream = ek(tc.tile_pool(name="stream", bufs=4))
        qkpool = ek(tc.tile_pool(name="qk", bufs=2))
        xvpool = ek(tc.tile_pool(name="xv", bufs=1))
        xvspool = ek(tc.tile_pool(name="xvs", bufs=4))
        attnpool = ek(tc.tile_pool(name="attn", bufs=10))
        sumpool = ek(tc.tile_pool(name="sums", bufs=8))
        ztpool = ek(tc.tile_pool(name="zt", bufs=1))
        outpool = ek(tc.tile_pool(name="out", bufs=3))
        pp_pf = ek(tc.tile_pool(name="pp_pf", bufs=2, space="PSUM"))
        pp_e = ek(tc.tile_pool(name="pp_e", bufs=4, space="PSUM"))
        pp_z = ek(tc.tile_pool(name="pp_z", bufs=1, space="PSUM"))

        # constants
        wq_sb = consts.tile([DX, DX], DT_MM, tag="wq")
        nc.sync.dma_start(wq_sb[:], wq[:])
        wk_sb = consts.tile([DX, DX], DT_MM, tag="wk")
        nc.sync.dma_start(wk_sb[:], wk[:])
        wcomb_sb = None
        bo_sb = None

        GH = 4  # heads per group (PSUM: one z bank per head pair)
        for s in range(S):
            # values for this sentence: [p, lc, e], l = lc*128 + p
            xv_sb = xvpool.tile([P, LC, E], F32, tag=f"xv{s % 2}")
            nc.sync.dma_start(xv_sb[:], xv[s].rearrange("(lo p) e -> p lo e", p=P))

            ZT = ztpool.tile([P, E // P, QP], DT_MM, tag=f"zt{s % 2}", name=f"zt_{s}")
            for g in range(H // GH):
                h0 = g * GH
                # projections: per head qT_ext/kT_ext; group q-projs then
                # k-projs so the stationary weight reloads only once.
                qes, kes = [], []
                xq_g = stream.tile([DX, GH, QP], DT_MM, tag="xq_g")
                nc.sync.dma_start(
                    xq_g[:], xq[s, h0:h0 + GH].rearrange("h d q -> d h q"))
                xk_g = stream.tile([DX, GH, L], DT_MM, tag="xk_g")
                nc.sync.dma_start(
                    xk_g[:], xk[s, h0:h0 + GH].rearrange("h d q -> d h q"))
                for i in range(GH):
                    h = h0 + i
                    pq = pp_pf.tile([DX, QP], F32, tag="pf", name="pq")
                    nc.tensor.matmul(pq[:], wq_sb[:], xq_g[:, i], start=True, stop=True)
                    qe = qkpool.tile([DX, QP], BF16, tag=f"qe{i}", name=f"qe_{s}_{h}")
                    nc.vector.tensor_copy(qe[:], pq[:])
                    qes.append(qe)
                for i in range(GH):
                    h = h0 + i
                    pk = pp_pf.tile([DX, L], F32, tag="pf", name="pk")
                    nc.tensor.matmul(pk[:], wk_sb[:], xk_g[:, i], start=True, stop=True)
                    ke = qkpool.tile([DX, L], BF16, tag=f"ke{i}", name=f"ke_{s}_{h}")
                    nc.vector.tensor_copy(ke[:], pk[:])
                    kes.append(ke)

                # one z psum bank per head PAIR: head A -> partitions 0:64,
                # head B -> partitions 64:128 (separate accumulation groups).
                zps = [
                    pp_z.tile([P, QP], F32, tag=f"z{p_}", name=f"zp_{s}_{g}_{p_}")
                    for p_ in range(GH // 2)
                ]
                for c in range(LC):
                    rsum = sumpool.tile([P, GH], F32, tag="rsum")
                    ats = []
                    for i in range(GH):
                        ep = pp_e.tile([P, QP], F32, tag="energy", name="ep")
                        nc.tensor.matmul(
                            ep[:],
                            kes[i][:, c * P:(c + 1) * P],
                            qes[i][:],
                            start=True,
                            stop=True,
                        )
                        at = attnpool.tile([P, QP], BF16, tag="at", name="at")
                        if i % 2 == 0:
                            # rowsum on the scalar engine (fused accumulate)
                            nc.scalar.activation(
                                at[:],
                                ep[:],
                                mybir.ActivationFunctionType.Exp,
                                scale=SCALE,
                                accum_out=rsum[:, i:i + 1],
                            )
                        else:
                            # rowsum on the vector engine (load balance)
                            nc.scalar.activation(
                                at[:],
                                ep[:],
                                mybir.ActivationFunctionType.Exp,
                                scale=SCALE,
                            )
                            nc.vector.tensor_reduce(
                                rsum[:, i:i + 1],
                                at[:],
                                axis=mybir.AxisListType.X,
                                op=mybir.AluOpType.add,
                            )
                        ats.append(at)
                    recip = sumpool.tile([P, GH], F32, tag="recip")
                    nc.vector.reciprocal(recip[:], rsum[:])
                    # xvs[p, i, d] = xv[p, c, (h0+i)*64+d] * recip[p, i]
                    xvs = xvspool.tile([P, GH, D], BF16, tag="xvs")
                    nc.vector.tensor_tensor(
                        xvs[:],
                        xv_sb[:, c, h0 * D:(h0 + GH) * D].rearrange(
                            "p (h d) -> p h d", d=D),
                        recip[:, :, None].to_broadcast((P, GH, D)),
                        mybir.AluOpType.mult,
                    )
                    for i in range(GH):
                        zp = zps[i // 2]
                        lo = (i % 2) * D
                        nc.tensor.matmul(
                            zp[lo:lo + D, :],
                            xvs[:, i],
                            ats[i][:],
                            start=(c == 0),
                            stop=(c == LC - 1),
                            skip_group_check=True,
                        )
                for p_ in range(GH // 2):
                    nc.vector.tensor_copy(ZT[:, g * (GH // 2) + p_, :], zps[p_][:])

            # fc_out: outT[j, l] = sum_e Wcomb[e, j] * ZT[e, l] + bo[j]
            if wcomb_sb is None:
                # emitted late so these big transfers don't delay the first
                # attention groups' input DMAs
                wcomb_sb = consts.tile([P, E // P, E], DT_MM, tag="wcomb")
                nc.sync.dma_start(
                    wcomb_sb[:], wcomb.rearrange("(eo p) j -> p eo j", p=P))
                bo_sb = consts.tile([P, E // P], F32, tag="bo")
                nc.sync.dma_start(bo_sb[:], bo.rearrange("(jo p) -> p jo", p=P))
            for jt in range(E // P):
                fp = pp_pf.tile([P, QP], F32, tag="pf", name="fp")
                for eo in range(E // P):
                    nc.tensor.matmul(
                        fp[:],
                        wcomb_sb[:, eo, jt * P:(jt + 1) * P],
                        ZT[:, eo, :],
                        start=(eo == 0),
                        stop=(eo == E // P - 1),
                    )
                ot = outpool.tile([P, QP], F32, tag="ot")
                nc.scalar.activation(
                    ot[:], fp[:], mybir.ActivationFunctionType.Identity,
                    bias=bo_sb[:, jt:jt + 1])
                nc.sync.dma_start(outT[s, jt * P:(jt + 1) * P, :], ot[:])


def host_prepare(values, keys, query, mask, Wv, Wk, Wq, Wo, bo):
    """Host-side sharding + layout + query compaction.

    Returns (in_maps, QP, order, cnt, bo_np). Masked query positions are
    dropped entirely (their output is just bo); the surviving queries are
    compacted to the front and padded to QP columns. Pad columns carry a
    -BMASK bias row so their exp is exactly 0 (excluded from denominators).
    """
    values = np.ascontiguousarray(np.asarray(values, dtype=np.float32))
    keys = np.asarray(keys, dtype=np.float32)
    query = np.asarray(query, dtype=np.float32)
    mask = np.asarray(mask)
    Wv = np.asarray(Wv, dtype=np.float32)
    Wk = np.asarray(Wk, dtype=np.float32)
    Wq = np.asarray(Wq, dtype=np.float32)
    Wo = np.asarray(Wo, dtype=np.float32)
    bo_np = np.ascontiguousarray(np.asarray(bo, dtype=np.float32))

    keep = mask[:, :, :, 0] != 0  # (N, S, L) True = query position survives
    cnt = keep.sum(-1)  # (N, S)
    QP = int(np.ceil(max(int(cnt.max()), 1) / 64) * 64)
    QP = max(QP, QP_MIN)
    QP = min(QP, L)
    # stable partition: surviving query indices first
    order = np.argsort(~keep, axis=-1, kind="stable")  # (N, S, L)

    qT = query.transpose(0, 1, 3, 2).reshape(N, S, H, D, L)
    kT = keys.transpose(0, 1, 3, 2).reshape(N, S, H, D, L)

    # gather+pad queries: (N, S, H, D, QP)
    gidx = order[:, :, :QP]  # (N, S, QP)
    qTc = np.take_along_axis(
        qT, gidx[:, :, None, None, :].repeat(H, 2).repeat(D, 3), axis=4)
    pad = np.arange(QP)[None, None, :] >= cnt[:, :, None]  # (N, S, QP)
    qTc[pad[:, :, None, None, :].repeat(H, 2).repeat(D, 3)] = 0.0
    qb_row = np.where(pad, np.float32(-BMASK), np.float32(0.0)).astype(np.float32)
    xq = np.ascontiguousarray(
        np.concatenate([qTc, qb_row[:, :, None, None, :].repeat(H, 2)], axis=3))

    ones_row = np.ones((N, S, H, 1, L), np.float32)
    xk = np.ascontiguousarray(np.concatenate([kT, ones_row], axis=3))

    wq_ext = np.zeros((DX, DX), np.float32)
    wq_ext[:D, :D] = Wq.T
    wq_ext[D, D] = 1.0
    wk_ext = np.zeros((DX, DX), np.float32)
    wk_ext[:D, :D] = Wk.T
    wk_ext[D, D] = 1.0

    wcomb = np.zeros((E, E), np.float32)
    for h in range(H):
        wcomb[h * D:(h + 1) * D, :] = Wv.T @ Wo[:, h * D:(h + 1) * D].T
    wcomb = np.ascontiguousarray(wcomb)

    shared = {"wq": wq_ext, "wk": wk_ext, "wcomb": wcomb, "bo": bo_np}
    in_maps = []
    for n in range(NCORES):
        m = {"xq": xq[n], "xk": xk[n], "xv": values[n]}
        m.update(shared)
        in_maps.append(m)
    return in_maps, QP, order, cnt, bo_np


_NC_CACHE = {}


def _get_program(QP):
    nc = _NC_CACHE.get(QP)
    if nc is not None:
        return nc
    nc = bacc.Bacc("TRN2", target_bir_lowering=False, debug=False,
                   num_devices=NCORES)
    ins = {
        "xq": nc.dram_tensor("xq", (S, H, DX, QP), DT_MM, kind="ExternalInput").ap(),
        "xk": nc.dram_tensor("xk", (S, H, DX, L), DT_MM, kind="ExternalInput").ap(),
        "xv": nc.dram_tensor("xv", (S, L, E), F32, kind="ExternalInput").ap(),
        "wq": nc.dram_tensor("wq", (DX, DX), DT_MM, kind="ExternalInput").ap(),
        "wk": nc.dram_tensor("wk", (DX, DX), DT_MM, kind="ExternalInput").ap(),
        "wcomb": nc.dram_tensor("wcomb", (E, E), DT_MM, kind="ExternalInput").ap(),
        "bo": nc.dram_tensor("bo", (E,), F32, kind="ExternalInput").ap(),
    }
    outs = {
        "outT": nc.dram_tensor("outT", (S, E, QP), F32, kind="ExternalOutput").ap(),
    }
    with tile.TileContext(nc) as tc:
        build_kernel_body(tc, outs, ins, QP)
    nc.compile()
    _NC_CACHE[QP] = nc
    return nc


def run(inputs: dict, trace: bool = False):
    """Run on 8 cores; returns (full_output, BassKernelResults)."""
    in_maps, QP, order, cnt, bo_np = host_prepare(**inputs)
    nc = _get_program(QP)
    res = bass_utils.run_bass_kernel_spmd(
        nc, in_maps, core_ids=list(range(NCORES)), trace=trace,
    )
    out = np.empty((N, S, L, E), np.float32)
    out[:] = bo_np  # masked query rows: attention output is 0, fc adds bo
    for n in range(NCORES):
        oT = res.results[n]["outT"]  # (S, E, QP)
        for s in range(S):
            c = int(cnt[n, s])
            if c:
                out[n, s, order[n, s, :c], :] = oT[s, :, :c].T
    return out, res


def kernel(**inputs) -> np.ndarray:
    out, _ = run(inputs, trace=False)
    return out
